# revision 9
# baseline (speedup 1.0000x reference)
"""BranchedLinear (block-diagonal grouped GEMM) Trainium2 kernel.

Reference computation:
    x:[N, 64*32] -> reshape [N, 64, 32];  out[n,b,:] = x[n,b,:] @ W[b] + bias[b]
    -> reshape [N, 64*32]

Strategy (8 NeuronCores, data-parallel on batch):
  * Shard batch N=16384 across 8 cores (2048 rows each).
  * The problem is HBM/fabric-bandwidth-bound. fp32 I/O moves 32 MiB/core;
    casting x and out to bf16 on the host halves that to 16 MiB/core at rel
    err ~3e-3 (gate 2e-2): inputs are quantized to 8-bit mantissas,
    products/accumulation stay exact in fp32 PSUM, bias is added in fp32,
    output rounds once to bf16. (fp8 x measures 2.6e-2 — over the gate.)
  * Host-side prep (numpy, cheap):
      - x shard is pre-transposed feature-major bf16: xt[g, p, n] =
        x[n, 128g+p] (g = 128-feature group of 4 branches). Every load is a
        512 KB DMA with contiguous 4 KB per-partition runs; the contraction
        dim (features) lands on SBUF partitions with no on-chip transpose.
      - W [64,32,32] is packed block-diagonal bf16 [128, 2048] (each 128-col
        group g holds branches 4g..4g+3 as 32x32 diagonal blocks), so a
        single K=128 matmul computes 4 branches at once.
      - bias is packed output-feature-major fp32 [128, 16].
  * Steady state is a paced pipeline: loads (SP HWDGE ring) recycle through
    6 strip buffers, stores (ACT ring) trail by the mm+copy latency, and the
    fabric services both directions (~420 GB/s combined when saturated). The
    per-strip rhythm is fabric-limited (~2.5 us = 1 MB in+out), so the PE
    must stay under that: ~12 throwaway warm-up matmuls run during the
    (otherwise idle) preamble to lift the PE out of the HAM cold state
    (4/8 clock = 1.2 GHz) before the first real matmul, and the steady
    rhythm then keeps it warm (idle gaps < 3.4 us).
  * Per (group g, chunk c) ONE bf16 matmul (single PE pass) with the
    block-diag W_g stationary and the 512-column x-transpose chunk moving;
    out.T [128 f_out, n] accumulates in fp32 PSUM.
  * The PSUM->SBUF copy + fp32 bias add is split between two engines: DVE
    (tensor_tensor, 1x mode ~1.2 us/half-strip) and ACT (activation
    Identity w/ per-partition bias, ~1.1 us), alternating per half-strip.
  * The first two groups' stores go out as 256 KB halves (primes the store
    queue early); the last group's too (short final drain + receipt).
"""

import numpy as np
import ml_dtypes

# Problem shape (hardcoded per contract)
BATCH = 16384
NUM_BRANCHES = 64
IN_FEATURES = 32
OUT_FEATURES = 32
D = NUM_BRANCHES * IN_FEATURES  # 2048

NUM_CORES = 8
SHARD = BATCH // NUM_CORES  # 2048 rows per core
P = 128
GROUPS = D // P  # 16 feature groups (4 branches each)
BRANCH_PER_GROUP = P // IN_FEATURES  # 4

CHUNK_N = 512  # matmul moving free dim (PSUM-bank limit at fp32 out)
HALF = SHARD // 2  # 1024: PSUM tile / copy granularity
WARMUP_MM = 12  # preamble matmuls to exit the HAM cold-clock state

USE_BF16 = True

_NC_CACHE = {}


def _np_io_dtype():
    return ml_dtypes.bfloat16 if USE_BF16 else np.float32


def _build_bass(use_bf16=USE_BF16):
    import concourse.mybir as mybir
    from concourse import bacc
    from concourse.tile import TileContext

    f32 = mybir.dt.float32
    fio = mybir.dt.bfloat16 if use_bf16 else f32
    shard = SHARD

    nc = bacc.Bacc("TRN2", target_bir_lowering=False, debug=False)
    xt = nc.dram_tensor("xt", [GROUPS, P, shard], fio, kind="ExternalInput")
    # host-packed block-diagonal [128, 2048]
    wbd = nc.dram_tensor("wbd", [P, D], fio, kind="ExternalInput")
    biasp = nc.dram_tensor("biasp", [P, GROUPS], f32, kind="ExternalInput")
    outp = nc.dram_tensor("outp", [GROUPS, P, shard], fio, kind="ExternalOutput")

    with TileContext(nc) as tc:
        with (
            tc.tile_pool(name="wpool", bufs=1) as wpool,
            tc.tile_pool(name="xpool", bufs=6) as xpool,
            tc.tile_pool(name="opool", bufs=4) as opool,
            tc.tile_pool(name="pspool", bufs=4, space="PSUM") as pspool,
        ):
            # W/bias ride the (otherwise idle at start) ACT store ring so the
            # first x strip isn't queued behind them on SP.
            b_sb = wpool.tile([P, GROUPS], f32, tag="b")
            nc.scalar.dma_start(out=b_sb[:], in_=biasp[:])
            w_sb = wpool.tile([P, D], fio, tag="w")
            nc.scalar.dma_start(out=w_sb[:], in_=wbd[:])

            # PE warm-up: throwaway matmuls on a zeroed tile while the DMAs
            # land. ~12 cold matmuls span ~4 us of PE-busy, enough for the
            # HAM activity window to release the clock gate before real work.
            junk = wpool.tile([P, CHUNK_N], fio, tag="junk")
            nc.vector.memset(junk[:], 0.0)
            psj = pspool.tile([P, HALF], f32, tag="ps", name="psj")
            for _ in range(WARMUP_MM):
                nc.tensor.matmul(
                    psj[:, :CHUNK_N], junk[:, :P], junk[:], start=True, stop=True
                )

            copy_idx = 0

            def psum_to_sbuf(dst, ps, g):
                # alternate the copy+bias between ACT and DVE
                nonlocal copy_idx
                if copy_idx % 2 == 0:
                    nc.scalar.activation(
                        dst,
                        ps,
                        mybir.ActivationFunctionType.Identity,
                        bias=b_sb[:, g : g + 1],
                    )
                else:
                    nc.vector.tensor_tensor(
                        dst,
                        ps,
                        b_sb[:, g : g + 1].to_broadcast((P, HALF)),
                        mybir.AluOpType.add,
                    )
                copy_idx += 1

            for g in range(GROUPS):
                xt_t = xpool.tile([P, shard], fio, tag="xt")
                nc.sync.dma_start(out=xt_t[:], in_=xt[:][g])
                o_t = opool.tile([P, shard], fio, tag="o")
                for h in range(2):
                    ps = pspool.tile([P, HALF], f32, tag="ps")
                    for ci in range(2):
                        c0 = h * HALF + ci * CHUNK_N
                        nc.tensor.matmul(
                            ps[:, ci * CHUNK_N : (ci + 1) * CHUNK_N],
                            w_sb[:, g * P : (g + 1) * P],
                            xt_t[:, c0 : c0 + CHUNK_N],
                            start=True,
                            stop=True,
                        )
                    dst = o_t[:, h * HALF : (h + 1) * HALF]
                    psum_to_sbuf(dst, ps[:], g)
                if g < 2 or g == GROUPS - 1:
                    # 256 KB halves: primes the store queue early (first
                    # groups) / keeps the final drain+receipt short (last)
                    for h in range(2):
                        nc.scalar.dma_start(
                            out=outp[:][g][:, h * HALF : (h + 1) * HALF],
                            in_=o_t[:, h * HALF : (h + 1) * HALF],
                        )
                elif g % 2 == 1:
                    # odd groups store via the (otherwise idle) GPSIMD SWDGE
                    # queue: store service isn't hostage to the load queue's
                    # arbitration priority
                    nc.gpsimd.dma_start(out=outp[:][g], in_=o_t[:])
                else:
                    nc.scalar.dma_start(out=outp[:][g], in_=o_t[:])
    nc.compile()
    return nc


def _get_nc(use_bf16=USE_BF16):
    key = (use_bf16,)
    if key not in _NC_CACHE:
        _NC_CACHE[key] = _build_bass(use_bf16)
    return _NC_CACHE[key]


def _pack_wbd(W):
    """[64, 32, 32] -> block-diagonal [128, 2048]."""
    W = np.asarray(W, np.float32)
    wbd = np.zeros((P, D), np.float32)
    for g in range(GROUPS):
        for j in range(BRANCH_PER_GROUP):
            b = g * BRANCH_PER_GROUP + j
            r0 = j * IN_FEATURES
            c0 = g * P + j * OUT_FEATURES
            wbd[r0 : r0 + IN_FEATURES, c0 : c0 + OUT_FEATURES] = W[b]
    return wbd.astype(_np_io_dtype())


def _pack_xt(shard):
    """[shard_n, 2048] -> [GROUPS, 128, shard_n] feature-major strips."""
    n = shard.shape[0]
    return np.ascontiguousarray(shard.T).astype(_np_io_dtype()).reshape(GROUPS, P, n)


def _pack_bias(b):
    """[64, 32] -> [128, GROUPS] output-feature-major."""
    return np.ascontiguousarray(np.asarray(b, np.float32).reshape(GROUPS, P).T)


def _unpack_out(outp):
    """[GROUPS, 128, shard_n] -> [shard_n, 2048] fp32."""
    n = outp.shape[-1]
    return outp.astype(np.float32).reshape(D, n).T


def kernel(x, W, b):
    from concourse.bass_utils import run_bass_kernel_spmd

    x = np.asarray(x, np.float32)
    wbd = _pack_wbd(W)
    biasp = _pack_bias(b)

    nc = _get_nc()
    in_maps = []
    for i in range(NUM_CORES):
        shard = x[i * SHARD : (i + 1) * SHARD]
        in_maps.append({"xt": _pack_xt(shard), "biasp": biasp, "wbd": wbd})

    res = run_bass_kernel_spmd(nc, in_maps, core_ids=list(range(NUM_CORES)))
    return np.ascontiguousarray(
        np.concatenate([_unpack_out(r["outp"]) for r in res.results], axis=0)
    )
